# revision 9
# baseline (speedup 1.0000x reference)
"""Nemotron/DeepSeek-style group-limited MoE router on 8 Trainium2 cores.

Full-input contract: kernel(**inputs) takes the full arrays
  hidden [16384, 4096] f32, weight [256, 4096] f32,
  router_bias [256] f32, scores_bias [256] f32
and returns (expert_idx int32 [16384, 8], weights f32 [16384, 8]).

Sharding: data-parallel over tokens. Each of the 8 cores routes 2048
tokens; the [256, 4096] router weight is replicated (transposed on host
so the contraction dim lands on SBUF partitions).

Per-core device kernel:
  logits = hidden_shard @ weight.T + router_bias   (PE, fp32, PSUM-resident
           accumulation over 32 K-chunks; bias seeded via a K=1 ones x bias
           matmul so no extra vector work)
  scores = sigmoid(logits) + scores_bias           (ACT + DVE)
  group-limited top-k on DVE: per-group top-8 (Max), top-2 sum, group
  threshold, masked top-8 (Max + MaxIndex), weight normalization.
"""

import numpy as np
from contextlib import ExitStack

import jax
from jax.sharding import Mesh, PartitionSpec
from jax.experimental.shard_map import shard_map

import concourse.bass as bass
import concourse.tile as tile
from concourse import bacc, mybir
from concourse import bass2jax

T, D, E = 16384, 4096, 256
N_CORES = 8
TPC = T // N_CORES          # tokens per core (2048)
N_GROUPS, TOPK = 8, 8
EPG = E // N_GROUPS         # experts per group (32)
SCALE = 2.5
KC = 128                    # contraction chunk (partition dim)
NKC = D // KC               # 32 K-chunks
N_TILES = TPC // 128        # 16 token tiles per core
QUARTERS = 4                # phases; 4 PSUM banks live per phase
TILES_PER_Q = N_TILES // QUARTERS    # 4
TOK_PER_Q = TPC // QUARTERS          # 512
CHUNKS_PER_DMA = 4          # K-chunks loaded per hidden DMA

USE_F32R = False            # fp32 matmul (exact); flip to use fast fp32r mode

_CACHED_NC = None


def _build_nc():
    f32 = mybir.dt.float32
    nc = bacc.Bacc("TRN2", target_bir_lowering=False, debug=False,
                   num_devices=N_CORES)

    hT = nc.dram_tensor("hT", [D, TPC], f32, kind="ExternalInput").ap()
    wT = nc.dram_tensor("wT", [D, E], f32, kind="ExternalInput").ap()
    rb = nc.dram_tensor("rb", [1, E], f32, kind="ExternalInput").ap()
    sb = nc.dram_tensor("sb", [1, E], f32, kind="ExternalInput").ap()
    idx_out = nc.dram_tensor("idx_out", [128, N_TILES * TOPK], mybir.dt.int32,
                             kind="ExternalOutput").ap()
    w_out = nc.dram_tensor("w_out", [128, N_TILES * TOPK], f32,
                           kind="ExternalOutput").ap()

    def mm_cast(ap):
        return ap.bitcast(mybir.dt.float32r) if USE_F32R else ap

    with tile.TileContext(nc) as tc:
        with ExitStack() as ctx:
            const = ctx.enter_context(tc.tile_pool(name="const", bufs=1))
            hpool = ctx.enter_context(tc.tile_pool(name="hid", bufs=3))
            spool = ctx.enter_context(tc.tile_pool(name="scores", bufs=4))
            tpool = ctx.enter_context(tc.tile_pool(name="topk", bufs=4))
            opool = ctx.enter_context(tc.tile_pool(name="outacc", bufs=1))
            psum = ctx.enter_context(
                tc.tile_pool(name="psum", bufs=2, space="PSUM"))

            # --- constants ---
            wT_s = const.tile([128, NKC * E], f32)        # all weight chunks
            for c in range(NKC):
                nc.gpsimd.dma_start(wT_s[:, c * E:(c + 1) * E],
                                    wT[c * KC:(c + 1) * KC, :])
            ones = const.tile([1, 128], f32)
            nc.vector.memset(ones[:], 1.0)
            rb_s = const.tile([1, E], f32)
            nc.gpsimd.dma_start(rb_s[:], rb[:])
            sb_s = const.tile([1, E], f32)
            nc.gpsimd.dma_start(sb_s[:], sb[:])

            # scores_bias broadcast to all 128 partitions via ones^T @ sb
            ps_sb = psum.tile([128, E], f32, name="ps_sb", tag="ps0")
            nc.tensor.matmul(ps_sb[:], ones[:], sb_s[:],
                             start=True, stop=True)
            sb_b = const.tile([128, E], f32)
            nc.scalar.activation(sb_b[:], ps_sb[:],
                                 mybir.ActivationFunctionType.Copy)

            idx_all = opool.tile([128, N_TILES * TOPK], mybir.dt.int32)
            w_all = opool.tile([128, N_TILES * TOPK], f32)

            for q in range(QUARTERS):
                # one full PSUM bank per token tile (4 live per phase)
                ps = [psum.tile([128, E], f32, name=f"ps{i}", tag=f"ps{i}")
                      for i in range(TILES_PER_Q)]

                def logit(m):
                    return ps[m][:]

                # seed accumulators with router_bias
                for m in range(TILES_PER_Q):
                    nc.tensor.matmul(logit(m), ones[:], rb_s[:],
                                     start=True, stop=False)

                # accumulate over K, 4 chunks per hidden DMA
                for g in range(NKC // CHUNKS_PER_DMA):
                    htile = hpool.tile([128, CHUNKS_PER_DMA * TOK_PER_Q], f32)
                    h3 = htile[:].rearrange("p (c f) -> p c f",
                                            c=CHUNKS_PER_DMA)
                    src = hT[g * CHUNKS_PER_DMA * KC:
                             (g + 1) * CHUNKS_PER_DMA * KC,
                             q * TOK_PER_Q:(q + 1) * TOK_PER_Q]
                    nc.gpsimd.dma_start(
                        h3, src.rearrange("(c p) f -> p c f", p=128))
                    for cl in range(CHUNKS_PER_DMA):
                        c = g * CHUNKS_PER_DMA + cl
                        last = (c == NKC - 1)
                        for m in range(TILES_PER_Q):
                            nc.tensor.matmul(
                                logit(m),
                                mm_cast(h3[:, cl, m * 128:(m + 1) * 128]),
                                mm_cast(wT_s[:, c * E:(c + 1) * E]),
                                start=False, stop=last)

                # epilogue per token tile
                for m in range(TILES_PER_Q):
                    mg = q * TILES_PER_Q + m
                    osl = slice(mg * TOPK, (mg + 1) * TOPK)

                    sig = spool.tile([128, E], f32)
                    nc.scalar.activation(sig[:], logit(m),
                                         mybir.ActivationFunctionType.Sigmoid)
                    nc.vector.tensor_add(sig[:], sig[:], sb_b[:])

                    g8 = tpool.tile([128, N_GROUPS * 8], f32)
                    for gr in range(N_GROUPS):
                        nc.vector.max(g8[:, gr * 8:(gr + 1) * 8],
                                      sig[:, gr * EPG:(gr + 1) * EPG])
                    g83 = g8[:].rearrange("p (g k) -> p g k", g=N_GROUPS)
                    gw = tpool.tile([128, N_GROUPS], f32)
                    nc.vector.tensor_add(gw[:], g83[:, :, 0], g83[:, :, 1])
                    gtop = tpool.tile([128, 8], f32)
                    nc.vector.max(gtop[:], gw[:])

                    # masked = (group_w >= 4th-largest) * scores
                    masked = spool.tile([128, E], f32)
                    nc.vector.scalar_tensor_tensor(
                        masked[:].rearrange("p (g e) -> p g e", g=N_GROUPS),
                        gw[:].unsqueeze(2).broadcast_to([128, N_GROUPS, EPG]),
                        gtop[:, 3:4],
                        sig[:].rearrange("p (g e) -> p g e", g=N_GROUPS),
                        op0=mybir.AluOpType.is_ge,
                        op1=mybir.AluOpType.mult)

                    v8 = tpool.tile([128, 8], f32)
                    nc.vector.max(v8[:], masked[:])
                    i8u = tpool.tile([128, 8], mybir.dt.uint32)
                    nc.vector.max_index(i8u[:], v8[:], masked[:])
                    nc.vector.tensor_copy(idx_all[:, osl], i8u[:])

                    ssum = tpool.tile([128, 1], f32)
                    nc.vector.tensor_reduce(ssum[:], v8[:],
                                            axis=mybir.AxisListType.X,
                                            op=mybir.AluOpType.add)
                    nc.vector.tensor_scalar_add(ssum[:], ssum[:], 1e-20)
                    rcp = tpool.tile([128, 1], f32)
                    nc.vector.reciprocal(rcp[:], ssum[:])
                    nc.vector.tensor_scalar(w_all[:, osl], v8[:], rcp[:],
                                            SCALE,
                                            op0=mybir.AluOpType.mult,
                                            op1=mybir.AluOpType.mult)

            nc.gpsimd.dma_start(idx_out[:], idx_all[:])
            nc.gpsimd.dma_start(w_out[:], w_all[:])

    nc.compile()
    return nc


_IN_NAMES = ["hT", "wT", "rb", "sb"]
_OUT_NAMES = ["idx_out", "w_out"]
_OUT_SHAPES = [(128, N_TILES * TOPK), (128, N_TILES * TOPK)]
_OUT_DTYPES = [np.int32, np.float32]

_CACHED = None  # (sharded_jit, mesh)


def _get_exec():
    """Build the SPMD PJRT executable once (compile happens on first call)."""
    global _CACHED
    if _CACHED is not None:
        return _CACHED
    bass2jax.install_neuronx_cc_hook()
    nc = _build_nc()
    out_avals = tuple(
        jax.core.ShapedArray(s, d) for s, d in zip(_OUT_SHAPES, _OUT_DTYPES))
    n_params = len(_IN_NAMES)
    n_outs = len(_OUT_NAMES)
    in_names = _IN_NAMES + _OUT_NAMES
    partition_name = (nc.partition_id_tensor.name
                      if nc.partition_id_tensor else None)
    if partition_name is not None:
        in_names = in_names + [partition_name]
    donate = tuple(range(n_params, n_params + n_outs))

    def _body(*args):
        operands = list(args)
        if partition_name is not None:
            operands.append(bass2jax.partition_id_tensor())
        outs = bass2jax._bass_exec_p.bind(
            *operands,
            out_avals=out_avals,
            in_names=tuple(in_names),
            out_names=tuple(_OUT_NAMES),
            lowering_input_output_aliases=(),
            sim_require_finite=True,
            sim_require_nnan=True,
            nc=nc,
        )
        return tuple(outs)

    devices = jax.devices()[:N_CORES]
    assert len(devices) == N_CORES
    mesh = Mesh(np.asarray(devices), ("core",))
    in_specs = (PartitionSpec("core"),) * (n_params + n_outs)
    out_specs = (PartitionSpec("core"),) * n_outs
    sharded = jax.jit(
        shard_map(_body, mesh=mesh, in_specs=in_specs, out_specs=out_specs,
                  check_rep=False),
        donate_argnums=donate, keep_unused=True)
    _CACHED = (sharded, mesh)
    return _CACHED


def prep_inputs(hidden, weight, router_bias, scores_bias):
    """Host-side shard prep: concat per-core inputs along axis 0."""
    hidden = np.asarray(hidden, dtype=np.float32)
    weight = np.asarray(weight, dtype=np.float32)
    router_bias = np.asarray(router_bias, dtype=np.float32)
    scores_bias = np.asarray(scores_bias, dtype=np.float32)

    # [D, T] with per-core column blocks -> concat of per-core [D, TPC]
    hT_all = np.concatenate(
        [np.ascontiguousarray(hidden[c * TPC:(c + 1) * TPC, :].T)
         for c in range(N_CORES)], axis=0)                 # [8*D, TPC]
    wT = np.ascontiguousarray(weight.T)                    # [D, E]
    wT_all = np.concatenate([wT] * N_CORES, axis=0)
    rb_all = np.broadcast_to(router_bias.reshape(1, E),
                             (N_CORES, E)).copy()
    sb_all = np.broadcast_to(scores_bias.reshape(1, E),
                             (N_CORES, E)).copy()
    return [hT_all, wT_all, rb_all, sb_all]


def _zero_outs():
    return [np.zeros((N_CORES * s[0], *s[1:]), d)
            for s, d in zip(_OUT_SHAPES, _OUT_DTYPES)]


def exec_concat(concat_in):
    sharded, _ = _get_exec()
    return sharded(*concat_in, *_zero_outs())


def unpack_outputs(out_arrs):
    idx_g = np.asarray(out_arrs[0]).reshape(N_CORES, 128, N_TILES, TOPK)
    w_g = np.asarray(out_arrs[1]).reshape(N_CORES, 128, N_TILES, TOPK)
    # [core, p, m, j] -> token = core*TPC + m*128 + p
    expert_idx = idx_g.transpose(0, 2, 1, 3).reshape(T, TOPK).astype(np.int32)
    weights = w_g.transpose(0, 2, 1, 3).reshape(T, TOPK).astype(np.float32)
    return expert_idx, weights


def kernel(hidden, weight, router_bias, scores_bias):
    concat_in = prep_inputs(hidden, weight, router_bias, scores_bias)
    out_arrs = exec_concat(concat_in)
    return unpack_outputs(out_arrs)


# revision 18
# speedup vs baseline: 769.6980x; 769.6980x over previous
"""Nemotron/DeepSeek-style group-limited MoE router on 8 Trainium2 cores.

Full-input contract: kernel(**inputs) takes the full arrays
  hidden [16384, 4096] f32, weight [256, 4096] f32,
  router_bias [256] f32, scores_bias [256] f32
and returns (expert_idx int32 [16384, 8], weights f32 [16384, 8]).

Sharding: data-parallel over tokens. Each of the 8 cores routes 2048
tokens; the [256, 4096] router weight is replicated (transposed on host
so the contraction dim lands on SBUF partitions).

Per-core device kernel:
  logits = hidden_shard @ weight.T + router_bias   (PE, fp32, PSUM-resident
           accumulation over 32 K-chunks; bias seeded via a K=1 ones x bias
           matmul so no extra vector work)
  scores = sigmoid(logits) + scores_bias           (ACT + DVE)
  group-limited top-k on DVE: per-group top-8 (Max), top-2 sum, group
  threshold, masked top-8 (Max + MaxIndex), weight normalization.
"""

import numpy as np
from contextlib import ExitStack

import jax
from jax.sharding import Mesh, PartitionSpec
from jax.experimental.shard_map import shard_map

import concourse.bass as bass
import concourse.tile as tile
from concourse import bacc, mybir
from concourse import bass2jax

T, D, E = 16384, 4096, 256
N_CORES = 8
TPC = T // N_CORES          # tokens per core (2048)
N_GROUPS, TOPK = 8, 8
EPG = E // N_GROUPS         # experts per group (32)
SCALE = 2.5
KC = 128                    # contraction chunk (partition dim)
NKC = D // KC               # 32 K-chunks
N_TILES = TPC // 128        # 16 token tiles per core
QUARTERS = 4                # phases; 4 PSUM banks live per phase
TILES_PER_Q = N_TILES // QUARTERS    # 4
TOK_PER_Q = TPC // QUARTERS          # 512
CHUNKS_PER_DMA = 4          # K-chunks loaded per hidden DMA

USE_F32R = False            # fp32 matmul (exact); flip to use fast fp32r mode


def _build_nc(loop_n=None):
    f32 = mybir.dt.float32
    mm_dt = mybir.dt.float32r if USE_F32R else f32
    nc = bacc.Bacc("TRN2", target_bir_lowering=False, debug=False,
                   num_devices=N_CORES)

    hT = nc.dram_tensor("hT", [D, TPC], mm_dt, kind="ExternalInput").ap()
    wT = nc.dram_tensor("wT", [D, E], mm_dt, kind="ExternalInput").ap()
    rb = nc.dram_tensor("rb", [1, E], f32, kind="ExternalInput").ap()
    sb = nc.dram_tensor("sb", [1, E], f32, kind="ExternalInput").ap()
    idx_out = nc.dram_tensor("idx_out", [128, N_TILES * TOPK], mybir.dt.int32,
                             kind="ExternalOutput").ap()
    w_out = nc.dram_tensor("w_out", [128, N_TILES * TOPK], f32,
                           kind="ExternalOutput").ap()

    with tile.TileContext(nc) as tc:
        with ExitStack() as ctx:
            const = ctx.enter_context(tc.tile_pool(name="const", bufs=1))
            hpool = ctx.enter_context(tc.tile_pool(name="hid", bufs=3))
            spool = ctx.enter_context(tc.tile_pool(name="scores", bufs=4))
            tpool = ctx.enter_context(tc.tile_pool(name="topk", bufs=4))
            opool = ctx.enter_context(tc.tile_pool(name="outacc", bufs=1))
            psum = ctx.enter_context(
                tc.tile_pool(name="psum", bufs=2, space="PSUM"))

            # --- constants ---
            wT_s = const.tile([128, NKC * E], mm_dt)      # all weight chunks
            for c in range(NKC):
                nc.gpsimd.dma_start(wT_s[:, c * E:(c + 1) * E],
                                    wT[c * KC:(c + 1) * KC, :])
            ones = const.tile([1, 128], f32)
            nc.vector.memset(ones[:], 1.0)
            rb_s = const.tile([1, E], f32)
            nc.gpsimd.dma_start(rb_s[:], rb[:])
            sb_s = const.tile([1, E], f32)
            nc.gpsimd.dma_start(sb_s[:], sb[:])

            # scores_bias broadcast to all 128 partitions via ones^T @ sb
            ps_sb = psum.tile([128, E], f32, name="ps_sb", tag="ps0")
            nc.tensor.matmul(ps_sb[:], ones[:], sb_s[:],
                             start=True, stop=True)
            sb_b = const.tile([128, E], f32)
            nc.scalar.activation(sb_b[:], ps_sb[:],
                                 mybir.ActivationFunctionType.Copy)

            idx_all = opool.tile([128, N_TILES * TOPK], mybir.dt.int32)
            w_all = opool.tile([128, N_TILES * TOPK], f32)

            loop_ctx = tc.For_i(0, loop_n, 1) if loop_n else None
            if loop_ctx is not None:
                loop_ctx.__enter__()

            for q in range(QUARTERS):
                # one full PSUM bank per token tile (4 live per phase)
                ps = [psum.tile([128, E], f32, name=f"ps{i}", tag=f"ps{i}")
                      for i in range(TILES_PER_Q)]

                def logit(m):
                    return ps[m][:]

                # seed accumulators with router_bias
                for m in range(TILES_PER_Q):
                    nc.tensor.matmul(logit(m), ones[:], rb_s[:],
                                     start=True, stop=False)

                # accumulate over K, 4 chunks per hidden DMA
                for g in range(NKC // CHUNKS_PER_DMA):
                    htile = hpool.tile([128, CHUNKS_PER_DMA * TOK_PER_Q],
                                       mm_dt)
                    h3 = htile[:].rearrange("p (c f) -> p c f",
                                            c=CHUNKS_PER_DMA)
                    src = hT[g * CHUNKS_PER_DMA * KC:
                             (g + 1) * CHUNKS_PER_DMA * KC,
                             q * TOK_PER_Q:(q + 1) * TOK_PER_Q]
                    nc.gpsimd.dma_start(
                        h3, src.rearrange("(c p) f -> p c f", p=128))
                    for cl in range(CHUNKS_PER_DMA):
                        c = g * CHUNKS_PER_DMA + cl
                        last = (c == NKC - 1)
                        for m in range(TILES_PER_Q):
                            nc.tensor.matmul(
                                logit(m),
                                h3[:, cl, m * 128:(m + 1) * 128],
                                wT_s[:, c * E:(c + 1) * E],
                                start=False, stop=last)

                # epilogue per token tile
                for m in range(TILES_PER_Q):
                    mg = q * TILES_PER_Q + m
                    osl = slice(mg * TOPK, (mg + 1) * TOPK)

                    sig = spool.tile([128, E], f32)
                    nc.scalar.activation(sig[:], logit(m),
                                         mybir.ActivationFunctionType.Sigmoid)
                    nc.vector.tensor_add(sig[:], sig[:], sb_b[:])

                    g8 = tpool.tile([128, N_GROUPS * 8], f32)
                    for gr in range(N_GROUPS):
                        nc.vector.max(g8[:, gr * 8:(gr + 1) * 8],
                                      sig[:, gr * EPG:(gr + 1) * EPG])
                    g83 = g8[:].rearrange("p (g k) -> p g k", g=N_GROUPS)
                    gw = tpool.tile([128, N_GROUPS], f32)
                    nc.vector.tensor_add(gw[:], g83[:, :, 0], g83[:, :, 1])
                    gtop = tpool.tile([128, 8], f32)
                    nc.vector.max(gtop[:], gw[:])

                    # masked = (group_w >= 4th-largest) * scores
                    masked = spool.tile([128, E], f32)
                    nc.vector.scalar_tensor_tensor(
                        masked[:].rearrange("p (g e) -> p g e", g=N_GROUPS),
                        gw[:].unsqueeze(2).broadcast_to([128, N_GROUPS, EPG]),
                        gtop[:, 3:4],
                        sig[:].rearrange("p (g e) -> p g e", g=N_GROUPS),
                        op0=mybir.AluOpType.is_ge,
                        op1=mybir.AluOpType.mult)

                    v8 = tpool.tile([128, 8], f32)
                    nc.vector.max(v8[:], masked[:])
                    i8u = tpool.tile([128, 8], mybir.dt.uint32)
                    nc.vector.max_index(i8u[:], v8[:], masked[:])
                    nc.vector.tensor_copy(idx_all[:, osl], i8u[:])

                    ssum = tpool.tile([128, 1], f32)
                    nc.vector.tensor_reduce(ssum[:], v8[:],
                                            axis=mybir.AxisListType.X,
                                            op=mybir.AluOpType.add)
                    nc.vector.tensor_scalar_add(ssum[:], ssum[:], 1e-20)
                    rcp = tpool.tile([128, 1], f32)
                    nc.vector.reciprocal(rcp[:], ssum[:])
                    nc.vector.tensor_scalar(w_all[:, osl], v8[:], rcp[:],
                                            SCALE,
                                            op0=mybir.AluOpType.mult,
                                            op1=mybir.AluOpType.mult)

            nc.gpsimd.dma_start(idx_out[:], idx_all[:])
            nc.gpsimd.dma_start(w_out[:], w_all[:])

            if loop_ctx is not None:
                loop_ctx.__exit__(None, None, None)

    nc.compile()
    return nc


_IN_NAMES = ["hT", "wT", "rb", "sb"]
_OUT_NAMES = ["idx_out", "w_out"]
_OUT_SHAPES = [(128, N_TILES * TOPK), (128, N_TILES * TOPK)]
_OUT_DTYPES = [np.int32, np.float32]

_CACHED = {}  # (USE_F32R, loop_n) -> (sharded_jit, mesh)


def _get_exec(loop_n=None):
    """Build the SPMD PJRT executable once (compile happens on first call)."""
    key = (USE_F32R, loop_n)
    if key in _CACHED:
        return _CACHED[key]
    bass2jax.install_neuronx_cc_hook()
    nc = _build_nc(loop_n=loop_n)
    out_avals = tuple(
        jax.core.ShapedArray(s, d) for s, d in zip(_OUT_SHAPES, _OUT_DTYPES))
    n_params = len(_IN_NAMES)
    n_outs = len(_OUT_NAMES)
    in_names = _IN_NAMES + _OUT_NAMES
    partition_name = (nc.partition_id_tensor.name
                      if nc.partition_id_tensor else None)
    if partition_name is not None:
        in_names = in_names + [partition_name]
    donate = tuple(range(n_params, n_params + n_outs))

    def _body(*args):
        operands = list(args)
        if partition_name is not None:
            operands.append(bass2jax.partition_id_tensor())
        outs = bass2jax._bass_exec_p.bind(
            *operands,
            out_avals=out_avals,
            in_names=tuple(in_names),
            out_names=tuple(_OUT_NAMES),
            lowering_input_output_aliases=(),
            sim_require_finite=True,
            sim_require_nnan=True,
            nc=nc,
        )
        return tuple(outs)

    devices = jax.devices()[:N_CORES]
    assert len(devices) == N_CORES
    mesh = Mesh(np.asarray(devices), ("core",))
    in_specs = (PartitionSpec("core"),) * (n_params + n_outs)
    out_specs = (PartitionSpec("core"),) * n_outs
    sharded = jax.jit(
        shard_map(_body, mesh=mesh, in_specs=in_specs, out_specs=out_specs,
                  check_rep=False),
        donate_argnums=donate, keep_unused=True)
    _CACHED[key] = (sharded, mesh)
    return _CACHED[key]


def _round_f32r(x):
    """Round fp32 to the bf16-pair representable set required by fp32r."""
    import ml_dtypes
    hi = x.astype(ml_dtypes.bfloat16).astype(np.float32)
    lo = (x - hi).astype(ml_dtypes.bfloat16).astype(np.float32)
    return hi + lo


def prep_inputs(hidden, weight, router_bias, scores_bias):
    """Host-side shard prep: concat per-core inputs along axis 0."""
    hidden = np.asarray(hidden, dtype=np.float32)
    weight = np.asarray(weight, dtype=np.float32)
    router_bias = np.asarray(router_bias, dtype=np.float32)
    scores_bias = np.asarray(scores_bias, dtype=np.float32)
    if USE_F32R:
        hidden = _round_f32r(hidden)
        weight = _round_f32r(weight)

    # [D, T] with per-core column blocks -> concat of per-core [D, TPC]
    hT_all = np.concatenate(
        [np.ascontiguousarray(hidden[c * TPC:(c + 1) * TPC, :].T)
         for c in range(N_CORES)], axis=0)                 # [8*D, TPC]
    wT = np.ascontiguousarray(weight.T)                    # [D, E]
    wT_all = np.concatenate([wT] * N_CORES, axis=0)
    rb_all = np.broadcast_to(router_bias.reshape(1, E),
                             (N_CORES, E)).copy()
    sb_all = np.broadcast_to(scores_bias.reshape(1, E),
                             (N_CORES, E)).copy()
    return [hT_all, wT_all, rb_all, sb_all]


def _zero_outs():
    return [np.zeros((N_CORES * s[0], *s[1:]), d)
            for s, d in zip(_OUT_SHAPES, _OUT_DTYPES)]


def exec_concat(concat_in, loop_n=None):
    sharded, _ = _get_exec(loop_n=loop_n)
    return sharded(*concat_in, *_zero_outs())


def unpack_outputs(out_arrs):
    idx_g = np.asarray(out_arrs[0]).reshape(N_CORES, 128, N_TILES, TOPK)
    w_g = np.asarray(out_arrs[1]).reshape(N_CORES, 128, N_TILES, TOPK)
    # [core, p, m, j] -> token = core*TPC + m*128 + p
    expert_idx = idx_g.transpose(0, 2, 1, 3).reshape(T, TOPK).astype(np.int32)
    weights = w_g.transpose(0, 2, 1, 3).reshape(T, TOPK).astype(np.float32)
    return expert_idx, weights


def kernel(hidden, weight, router_bias, scores_bias):
    concat_in = prep_inputs(hidden, weight, router_bias, scores_bias)
    out_arrs = exec_concat(concat_in)
    return unpack_outputs(out_arrs)
